# revision 15
# baseline (speedup 1.0000x reference)
"""DeepseekV4-style sparse attention on 8 Trainium2 cores (Bass/Tile).

Sharding: data-parallel over batch (2) x tensor-parallel over heads (16 -> 4
groups of 4).  Core c handles batch c//4 and heads [4*(c%4), 4*(c%4)+4).
Per-core partial outputs (attn_heads @ wo_rows) are summed on the host
(partials are emitted bf16; the host accumulates in f64).

The shared-KV work (kv/gate projection + gated pooling) is sharded across the
4 head-group cores of each batch: core g projects/pools only its quarter of
the sequence (1024 positions -> 256 windows), then an intra-batch AllGather
(replica groups [[0..3],[4..7]]) shares the finished bf16 kT / V tiles.  The
collective rides entirely behind the q projection (~120us of PE work).

Device layout notes:
  - host passes hidden TRANSPOSED ([HID, S]) and cast to bf16 so every matmul
    has its contraction dim on partitions with no on-device transposes
  - hq is the per-core kv/gate slab [HID, 1028]: columns = positions
    [1024g-4, 1024g+1024) (4-col halo for the cross-window pooling overlap;
    core g=0 gets zeros + a -30000 additive gate pad instead)
  - weights/tables load ONCE (outside the unrolled timing reps)
  - q is produced directly in qT layout [head_dim, t] (matmul lhsT = wq)
  - scores are computed transposed (S^T[w, q]); the softmax denominator is
    accumulated by an ALL-ONES [128,128] lhsT so it lands replicated on all
    128 PSUM partitions; 1/(den+esink) runs as ACT ln -> exp(-x)
  - causal structure: query chunk j (512 queries) sees w-chunks 0..j; only
    the diagonal chunk needs a mask, identical for every j (precomputed 0/1)
  - out-proj for chunk j is emitted after chunk j+1's attention so its
    attnT-wait never blocks the strictly-ordered PE queue
  - all RoPE math runs on partitions 64..127 so no op shifts partition bases
"""

import numpy as np
import ml_dtypes

import concourse.bass as bass
import concourse.mybir as mybir
import concourse.tile as tile
from concourse.bass import ts
from concourse.masks import make_identity

F32 = mybir.dt.float32
BF16 = mybir.dt.bfloat16
AF = mybir.ActivationFunctionType

# Problem constants (hardcoded per the harness contract).
B, S, HID, NH, HD, RD, RATIO = 2, 4096, 2048, 16, 128, 64, 4
THETA = 10000.0
NW = S // RATIO              # 1024 pooled windows
N_CORES = 8
HPC = 4                      # heads per core
CW = HPC * HD                # per-core q/wo width (512)
TCH = 512                    # t-chunk size
NCH = S // TCH               # 8 t-chunks
WCH = 128                    # w-chunk size
KCH = HID // 128             # 16 contraction chunks
SCALE = HD ** -0.5
NWL = NW // HPC              # 256 windows pooled locally per core
NCHL = (NWL * RATIO) // TCH  # 2 local kv/gate chunks
HQW = NWL * RATIO + RATIO    # 1028 (quarter + halo)
RG = [[0, 1, 2, 3], [4, 5, 6, 7]]   # intra-batch allgather groups

_PAIR_SWAP = [i ^ 1 for i in range(32)]


def _build_nc(n_reps: int = 1, split_waits: bool = True):
    nc = bass.Bass()
    dp = nc.declare_dram_parameter
    ht = dp("ht", [HID, S], BF16, isOutput=False)
    hq = dp("hq", [HID, HQW], BF16, isOutput=False)
    wq = dp("wq", [HID, CW], BF16, isOutput=False)
    wkv = dp("wkv", [HID, 2 * HD], BF16, isOutput=False)
    wg = dp("wg", [HID, 2 * HD], BF16, isOutput=False)
    wo = dp("wo", [CW, HID], BF16, isOutput=False)
    eape = dp("eape", [HD, 2 * RATIO], F32, isOutput=False)
    esink = dp("esink", [128, HPC], F32, isOutput=False)
    gpad = dp("gpad", [HD, RATIO], F32, isOutput=False)
    cosq = dp("cosq", [RD, S], BF16, isOutput=False)
    sinq = dp("sinq", [RD, S], BF16, isOutput=False)
    coskl = dp("coskl", [RD, NWL], BF16, isOutput=False)
    sinkl = dp("sinkl", [RD, NWL], BF16, isOutput=False)
    bandm = dp("bandm", [WCH, TCH], BF16, isOutput=False)
    out = dp("out", [S, HID], BF16, isOutput=True)

    with tile.TileContext(nc) as tc:
        with (
            tc.tile_pool(name="persist", bufs=1) as pp,
            tc.tile_pool(name="wts", bufs=1) as wts,
            tc.tile_pool(name="ccd", bufs=1, space="DRAM") as ccd,
        ):
            P = _preamble(nc, pp, wts, ccd, wq, wkv, wg, wo, eape, esink,
                          gpad, coskl, sinkl, bandm)
            # NOTE: collectives inside a hardware For_i loop desync the NRT
            # mesh ("rolled collectives"), so timing builds unroll the rep
            # python-side: every rep gets its own straight-line CC instance
            # (the same way the single-rep program executes repeatedly).
            for _ in range(n_reps):
                _rep(nc, tc, P, ht, hq, cosq, sinq, out)
    if split_waits:
        _split_multi_waits(nc)
    return nc


def _preamble(nc, pp, wts, ccd, wq, wkv, wg, wo, eape, esink, gpad,
              coskl, sinkl, bandm):
    """Weights, tables and constants: loaded once, live across reps."""
    P = {}
    P["qT"] = [pp.tile([128, S], BF16, tag=f"qT{m}", name=f"qT{m}")
               for m in range(HPC)]
    # local kv/gate slabs: the 4-col halo shift is baked into the slab
    P["kvlo"] = pp.tile([HD, NWL * RATIO], BF16, tag="kvlo", name="kvlo")
    P["kvhi"] = pp.tile([HD, NWL * RATIO], BF16, tag="kvhi", name="kvhi")
    P["glo"] = pp.tile([HD, NWL * RATIO], BF16, tag="glo", name="glo")
    P["ghi"] = pp.tile([HD, NWL * RATIO], BF16, tag="ghi", name="ghi")
    # k-rope tables live on partitions 64..127 (matching the rope rows)
    P["cosk_s"] = pp.tile([128, NWL], BF16, tag="cosk", name="cosk")
    P["sink_s"] = pp.tile([128, NWL], BF16, tag="sink", name="sink")
    P["eape_s"] = pp.tile([HD, 2 * RATIO], F32, tag="eape", name="eape")
    P["esink_s"] = pp.tile([128, HPC], F32, tag="esink", name="esink")
    P["gpad_s"] = pp.tile([HD, RATIO], F32, tag="gpad", name="gpad")
    P["bandm_s"] = pp.tile([WCH, TCH], BF16, tag="bandm", name="bandm")
    P["wo_s"] = pp.tile([HD, HPC, HID], BF16, tag="wo", name="wo")
    P["ones128"] = pp.tile([WCH, 128], BF16, tag="ones128", name="ones128")
    P["kT"] = pp.tile([HD, NW], BF16, tag="kT", name="kT")
    P["v_s"] = pp.tile([WCH, NW // WCH, HD], BF16, tag="v", name="v")
    P["pooledT"] = pp.tile([HD, NWL], F32, tag="pooledT", name="pooledT")
    P["pk"] = pp.tile([128, 2 * NWL], BF16, tag="pk", name="pk")
    P["ident"] = pp.tile([128, 128], F32, tag="ident", name="ident")
    P["wq_s"] = wts.tile([128, KCH, CW], BF16, tag="wq", name="wq")
    P["wkv_s"] = wts.tile([128, KCH, 2 * HD], BF16, tag="wkv", name="wkv")
    P["wg_s"] = wts.tile([128, KCH, 2 * HD], BF16, tag="wg", name="wg")
    P["ccin"] = ccd.tile([128, 2 * NWL], BF16, tag="ccin", name="ccin")
    P["ccout"] = ccd.tile([HPC * 128, 2 * NWL], BF16, tag="ccout",
                          name="ccout")

    nc.sync.dma_start(P["wq_s"][:], wq.rearrange("(k p) c -> p k c", p=128))
    nc.sync.dma_start(P["wkv_s"][:], wkv.rearrange("(k p) c -> p k c", p=128))
    nc.sync.dma_start(P["wg_s"][:], wg.rearrange("(k p) c -> p k c", p=128))
    nc.sync.dma_start(P["cosk_s"][RD:128, :], coskl[:])
    nc.sync.dma_start(P["sink_s"][RD:128, :], sinkl[:])
    nc.sync.dma_start(P["eape_s"][:], eape[:])
    nc.sync.dma_start(P["esink_s"][:], esink[:])
    nc.sync.dma_start(P["gpad_s"][:], gpad[:])
    nc.sync.dma_start(P["bandm_s"][:], bandm[:])
    nc.sync.dma_start(P["wo_s"][:], wo.rearrange("(h p) e -> p h e", p=HD))
    nc.vector.memset(P["ones128"][:], 1.0)
    make_identity(nc, P["ident"][:])
    return P


def _rep(nc, tc, P, ht, hq, cosq, sinq, out):
    qT, kvlo, kvhi, glo, ghi = (P["qT"], P["kvlo"], P["kvhi"], P["glo"],
                                P["ghi"])
    kT, v_s, pooledT, pk = P["kT"], P["v_s"], P["pooledT"], P["pk"]
    eape_s, esink_s, bandm_s, wo_s = (P["eape_s"], P["esink_s"], P["bandm_s"],
                                      P["wo_s"])
    ones128, ident, gpad_s = P["ones128"], P["ident"], P["gpad_s"]
    wq_s, wkv_s, wg_s = P["wq_s"], P["wkv_s"], P["wg_s"]
    ccin, ccout = P["ccin"], P["ccout"]

    # ---- stage A: local kv/gate proj + pooling + allgather, q proj ----
    with (
        tc.tile_pool(name="hslab", bufs=2) as hs,
        tc.tile_pool(name="hqslab", bufs=1) as hqp,
        tc.tile_pool(name="evict", bufs=3) as ev,
        tc.tile_pool(name="poolb", bufs=2) as pb,
        tc.tile_pool(name="poolacc", bufs=1) as pba,
        tc.tile_pool(name="psA", bufs=6, space="PSUM") as psA,
        tc.tile_pool(name="psT", bufs=2, space="PSUM") as psT,
    ):
        numer = pba.tile([HD, NWL], F32, tag="numer", name="numer")
        denom = pba.tile([HD, NWL], F32, tag="denom", name="denom")

        def _kv_gate_local(jc, hqs):
            # kv / gate projections of the local quarter (lo = features
            # 0:128 at halo offset 0, hi = 128:256 at offset RATIO); the
            # gate eviction folds in the additive pooling bias ape
            for dst, lo, wsrc in (
                (kvlo, True, wkv_s), (kvhi, False, wkv_s),
                (glo, True, wg_s), (ghi, False, wg_s),
            ):
                ps = psA.tile([128, TCH], F32, tag="psA", name="psA")
                col = slice(0, 128) if lo else slice(128, 256)
                base = jc * TCH + (0 if lo else RATIO)
                for k in range(KCH):
                    nc.tensor.matmul(ps[:], wsrc[:, k, col],
                                     hqs[:, k, base:base + TCH],
                                     start=(k == 0), stop=(k == KCH - 1))
                dsl = dst[:, ts(jc, TCH)]
                if dst is kvlo or dst is kvhi:
                    nc.scalar.copy(dsl, ps[:])
                else:
                    acol = slice(0, RATIO) if lo else slice(RATIO, 2 * RATIO)
                    nc.vector.tensor_add(
                        dsl.rearrange("d (w r) -> d w r", r=RATIO),
                        ps[:].rearrange("d (w r) -> d w r", r=RATIO),
                        eape_s[:, None, acol].to_broadcast(
                            [HD, TCH // RATIO, RATIO]))

        def _pool_chunk(jc):
            # pool the 128 windows [128jc, 128jc+128) from chunk jc
            wsl = ts(jc, TCH // RATIO)
            for half, (g_src, kv_src) in enumerate(
                ((glo, kvlo), (ghi, kvhi))
            ):
                csl = ts(jc, TCH)
                e = pb.tile([HD, TCH], BF16, tag="poole", name=f"poole{half}")
                nc.scalar.activation(e[:], g_src[:, csl], AF.Exp)
                ea = pb.tile([HD, TCH], F32, tag="poolea",
                             name=f"poolea{half}")
                nc.vector.tensor_mul(ea[:], e[:], kv_src[:, csl])
                for acc, src in ((denom, e), (numer, ea)):
                    s3 = src[:].rearrange("d (w r) -> d w r", r=RATIO)
                    nm = f"poolred{half}{1 if acc is numer else 0}"
                    ra = pb.tile([HD, TCH // RATIO], F32, tag="poolra",
                                 name=nm + "a")
                    nc.vector.tensor_add(ra[:], s3[:, :, 0], s3[:, :, 1])
                    rc = pb.tile([HD, TCH // RATIO], F32, tag="poolrc",
                                 name=nm + "c")
                    nc.vector.tensor_add(rc[:], s3[:, :, 2], s3[:, :, 3])
                    if half == 0:
                        nc.vector.tensor_add(acc[:, wsl], ra[:], rc[:])
                    else:
                        nc.vector.tensor_add(ra[:], ra[:], rc[:])
                        nc.vector.tensor_add(acc[:, wsl], acc[:, wsl], ra[:])

        def _pool_fin_local():
            # divide, RoPE, V-transpose -> pack tile pk = [kT_local | v_loc]
            rec = pb.tile([HD, NWL], F32, tag="poolrec", name="poolrec")
            nc.vector.reciprocal(rec[:], denom[:])
            nc.vector.tensor_mul(pooledT[:], numer[:], rec[:])
            nc.scalar.copy(pk[0:RD, 0:NWL], pooledT[0:RD, :])
            krb = pb.tile([128, NWL], BF16, tag="krope", name="krope")
            nc.scalar.copy(krb[RD:128, :], pooledT[RD:128, :])
            ksw = pb.tile([128, NWL], BF16, tag="kswap", name="kswap")
            nc.vector.stream_shuffle(ksw[RD:128, :], krb[RD:128, :],
                                     _PAIR_SWAP)
            nc.vector.tensor_mul(krb[RD:128, :], krb[RD:128, :],
                                 P["cosk_s"][RD:128, :])
            nc.vector.tensor_mul(ksw[RD:128, :], ksw[RD:128, :],
                                 P["sink_s"][RD:128, :])
            nc.vector.tensor_add(pk[RD:128, 0:NWL], krb[RD:128, :],
                                 ksw[RD:128, :])
            for wb in range(NWL // WCH):
                tp = psT.tile([128, 128], F32, tag="vtrans", name="vtrans")
                nc.tensor.transpose(tp[:], pooledT[:, ts(wb, 128)], ident[:])
                nc.scalar.copy(pk[:, NWL + wb * 128:NWL + (wb + 1) * 128],
                               tp[:])

        def _q_proj(j, hsl):
            tsl = ts(j, TCH)
            cq_t = ev.tile([128, TCH], BF16, tag="cosqc", name="cosqc")
            sq_t = ev.tile([128, TCH], BF16, tag="sinqc", name="sinqc")
            nc.sync.dma_start(cq_t[RD:128, :], cosq[:, tsl])
            nc.sync.dma_start(sq_t[RD:128, :], sinq[:, tsl])
            for m in range(HPC):
                ps = psA.tile([128, TCH], F32, tag="psA", name="psA")
                for k in range(KCH):
                    nc.tensor.matmul(ps[:], wq_s[:, k, ts(m, 128)],
                                     hsl[:, k, :], start=(k == 0),
                                     stop=(k == KCH - 1))
                # rows 0:64 pass through; rows 64:128 interleaved RoPE
                nc.scalar.copy(qT[m][0:RD, tsl], ps[0:RD, :])
                rb = ev.tile([128, TCH], BF16, tag="ropebuf", name="ropebuf")
                nc.scalar.copy(rb[RD:128, :], ps[RD:128, :])
                sw = ev.tile([128, TCH], BF16, tag="ropeswap", name="ropeswap")
                nc.vector.stream_shuffle(sw[RD:128, :], rb[RD:128, :],
                                         _PAIR_SWAP)
                t1 = ev.tile([128, TCH], BF16, tag="ropet1", name="ropet1")
                nc.vector.tensor_mul(t1[RD:128, :], rb[RD:128, :],
                                     cq_t[RD:128, :])
                t2 = ev.tile([128, TCH], BF16, tag="ropet2", name="ropet2")
                nc.vector.tensor_mul(t2[RD:128, :], sw[RD:128, :],
                                     sq_t[RD:128, :])
                nc.vector.tensor_add(qT[m][RD:128, tsl], t1[RD:128, :],
                                     t2[RD:128, :])

        # q chunk 0 first so the PE has work while the bigger hq slab lands
        hsl0 = hs.tile([128, KCH, TCH], BF16, tag="hslab", name="hslab")
        nc.sync.dma_start(
            hsl0[:], ht[:, ts(0, TCH)].rearrange("(k p) t -> p k t", p=128))
        hqs = hqp.tile([128, KCH, HQW], BF16, tag="hqslab", name="hqslab")
        nc.sync.dma_start(hqs[:], hq.rearrange("(k p) c -> p k c", p=128))
        _q_proj(0, hsl0)
        for jc in range(NCHL):
            _kv_gate_local(jc, hqs)
        # additive gate pad for the first window's halo (core g=0 only;
        # other cores receive zeros)
        nc.vector.tensor_add(glo[:, 0:RATIO], glo[:, 0:RATIO], gpad_s[:])
        for jc in range(NCHL):
            _pool_chunk(jc)
        _pool_fin_local()
        nc.sync.dma_start(ccin[:], pk[:])
        nc.gpsimd.collective_compute(
            "AllGather", mybir.AluOpType.bypass, replica_groups=RG,
            ins=[ccin[:]], outs=[ccout[:]])
        nc.sync.dma_start(
            kT[:].rearrange("p (g w) -> p g w", g=HPC),
            ccout[:, 0:NWL].rearrange("(g p) w -> p g w", g=HPC))
        nc.sync.dma_start(
            v_s[:].rearrange("p (g wb) d -> p g wb d", g=HPC),
            ccout[:, NWL:2 * NWL].rearrange("(g p) (wb d) -> p g wb d",
                                            g=HPC, d=HD))
        for j in range(1, NCH):
            hsl = hs.tile([128, KCH, TCH], BF16, tag="hslab", name="hslab")
            nc.sync.dma_start(
                hsl[:], ht[:, ts(j, TCH)].rearrange("(k p) t -> p k t", p=128))
            _q_proj(j, hsl)

    # ---- stage C: attention + output projection, per q-chunk ----
    with (
        tc.tile_pool(name="pt", bufs=4) as ptp,
        tc.tile_pool(name="att", bufs=2) as att,
        tc.tile_pool(name="osb", bufs=6) as osb,
        tc.tile_pool(name="psS", bufs=2, space="PSUM") as psS,
        tc.tile_pool(name="psO", bufs=2, space="PSUM") as pso,
        tc.tile_pool(name="psAcc", bufs=2, space="PSUM") as psacc,
    ):
        def _attn_chunk(j):
            tsl = ts(j, TCH)
            attnT = []
            for h in range(HPC):
                qsl = qT[h][:, tsl]
                den_ps = psacc.tile([128, TCH], F32, tag="den", name="den")
                acc_ps = psacc.tile([128, TCH], F32, tag="acc", name="acc")
                # score matmuls run one w-chunk ahead of the exp/sum/PV
                # consumers so PE never sits behind the ACT exp
                s_tiles = {}

                def _score(wc, s_tiles=s_tiles, qsl=qsl):
                    sp = psS.tile([WCH, TCH], F32, tag="s", name="s")
                    nc.tensor.matmul(sp[:], kT[:, ts(wc, WCH)], qsl,
                                     start=True, stop=True)
                    s_tiles[wc] = sp

                _score(0)
                for wc in range(j + 1):
                    if wc < j:
                        _score(wc + 1)
                    s_ps = s_tiles.pop(wc)
                    pt = ptp.tile([WCH, TCH], BF16, tag="pt", name="pt")
                    nc.scalar.activation(pt[:], s_ps[:], AF.Exp, scale=SCALE)
                    if wc == j:
                        ptm = ptp.tile([WCH, TCH], BF16, tag="ptm", name="ptm")
                        nc.vector.tensor_mul(ptm[:], pt[:], bandm_s[:])
                        pt = ptm
                    # denominator rides on an all-ones lhsT so it lands
                    # replicated across all 128 PSUM partitions
                    nc.tensor.matmul(den_ps[:], ones128[:], pt[:],
                                     start=(wc == 0), stop=(wc == j))
                    nc.tensor.matmul(acc_ps[:], v_s[:, wc, :], pt[:],
                                     start=(wc == 0), stop=(wc == j))
                # 1/(den+esink) via ACT ln->exp(-x): keeps the 3.4us DVE
                # reciprocal off the critical path (ACT is lightly loaded)
                lden = att.tile([128, TCH], F32, tag="lden", name="lden")
                nc.scalar.activation(lden[:], den_ps[:], AF.Ln,
                                     bias=esink_s[:, h:h + 1])
                rec_sb = att.tile([128, TCH], F32, tag="rec_sb", name="rec_sb")
                nc.scalar.activation(rec_sb[:], lden[:], AF.Exp, scale=-1.0)
                a_sb = att.tile([128, TCH], BF16, tag=f"attnT{h}",
                                name=f"attnT{h}")
                nc.vector.tensor_mul(a_sb[:], acc_ps[:], rec_sb[:])
                attnT.append(a_sb)
            return attnT

        def _out_proj(j, attnT):
            for tt in range(TCH // 128):
                for e in range(HID // TCH):
                    o_ps = pso.tile([128, TCH], F32, tag="o", name="o")
                    for h in range(HPC):
                        nc.tensor.matmul(o_ps[:], attnT[h][:, ts(tt, 128)],
                                         wo_s[:, h, ts(e, TCH)],
                                         start=(h == 0), stop=(h == HPC - 1))
                    o_sb = osb.tile([128, TCH], BF16, tag="o_sb", name="o_sb")
                    # DVE, not ACT: the strict-FIFO ACT queue carries the
                    # score exps; a copy queued there delays the next head's
                    # den/acc matmuls
                    nc.vector.tensor_copy(o_sb[:], o_ps[:])
                    nc.sync.dma_start(
                        out[j * TCH + tt * 128:j * TCH + (tt + 1) * 128,
                            ts(e, TCH)], o_sb[:])

        # out-proj for a chunk is emitted AFTER the next chunk's attention so
        # its attnT-wait never blocks the later score matmuls in the
        # strictly-ordered PE queue (attnT tags are double-buffered).
        # Descending j: the small causal chunks run last, where the previous
        # chunk's pipelined out-proj fills their epilogue stalls.
        prev = None
        for j in range(NCH - 1, -1, -1):
            cur = (j, _attn_chunk(j))
            if prev is not None:
                _out_proj(*prev)
            prev = cur
        _out_proj(*prev)


_WS_CTR = [0]


def _split_multi_waits(nc):
    """This walrus build accepts at most ONE sync wait per instruction; hoist
    extras onto same-engine NOPs placed immediately before."""
    f = nc.m.functions[0]
    for blk in f.blocks:
        insts = blk.instructions
        if not any(i.sync_info is not None and len(i.sync_info.on_wait) > 1
                   for i in insts):
            continue
        new_list = []
        for inst in insts:
            si = inst.sync_info
            if si is not None and len(si.on_wait) > 1:
                waits = list(si.on_wait)
                for w in waits[:-1]:
                    _WS_CTR[0] += 1
                    new_list.append(mybir.InstNoOp(
                        name=f"waitsplit-{_WS_CTR[0]}",
                        engine=inst.engine,
                        bass_nofuse=True,
                        sync_info=mybir.SyncInfo(on_wait=[w], on_update=[])))
                inst.sync_info = mybir.SyncInfo(
                    on_wait=[waits[-1]], on_update=list(si.on_update))
            new_list.append(inst)
        blk.instructions = new_list


# ---------------------------------------------------------------------------
# host side
# ---------------------------------------------------------------------------

def _rope_tables(positions):
    half = RD // 2
    inv_freq = 1.0 / (THETA ** (np.arange(half, dtype=np.float64) / half))
    ang = positions[None, :].astype(np.float64) * inv_freq[:, None]  # [32, L]
    cos_t = np.repeat(np.cos(ang), 2, axis=0).astype(np.float32)
    sin_t = np.repeat(np.sin(ang), 2, axis=0).astype(np.float32)
    sin_t[0::2] *= -1.0                                  # a-rows get -sin
    return cos_t, sin_t


def _prep_inputs(hidden, wq, wkv, wgate, ape, sinks, wo):
    bf = ml_dtypes.bfloat16
    cosq_t, sinq_t = _rope_tables(np.arange(S))
    cosk_t, sink_t = _rope_tables(np.arange(NW) * RATIO)
    pw, ft = np.meshgrid(np.arange(WCH), np.arange(TCH), indexing="ij")
    band = (ft >= RATIO * pw + RATIO - 1).astype(np.float32)     # [WCH, TCH]
    eape = np.empty((HD, 2 * RATIO), np.float32)
    for r in range(RATIO):
        eape[:, r] = ape[r, :HD]
        eape[:, RATIO + r] = ape[r, HD:]
    maps = []
    for c in range(N_CORES):
        b, g = divmod(c, HPC)
        htb = np.ascontiguousarray(hidden[b].T).astype(bf)
        if g == 0:
            hqm = np.concatenate(
                [np.zeros((HID, RATIO), bf), htb[:, :NWL * RATIO]], axis=1)
            gp = np.full((HD, RATIO), -30000.0, np.float32)
        else:
            lo = g * NWL * RATIO - RATIO
            hqm = htb[:, lo:lo + HQW]
            gp = np.zeros((HD, RATIO), np.float32)
        maps.append({
            "ht": htb,
            "hq": np.ascontiguousarray(hqm),
            "wq": np.ascontiguousarray(wq[:, g * CW:(g + 1) * CW]).astype(bf),
            "wkv": wkv.astype(bf),
            "wg": wgate.astype(bf),
            "wo": np.ascontiguousarray(wo[g * CW:(g + 1) * CW, :]).astype(bf),
            "eape": eape,
            "esink": np.tile(
                np.exp(sinks[g * HPC:(g + 1) * HPC]).astype(
                    np.float32).reshape(1, HPC), (128, 1)),
            "gpad": gp,
            "cosq": cosq_t.astype(bf), "sinq": sinq_t.astype(bf),
            "coskl": np.ascontiguousarray(
                cosk_t[:, g * NWL:(g + 1) * NWL]).astype(bf),
            "sinkl": np.ascontiguousarray(
                sink_t[:, g * NWL:(g + 1) * NWL]).astype(bf),
            "bandm": band.astype(bf),
        })
    return maps


_RUNNER_CACHE = {}


def _get_runner(n_reps: int = 1):
    if n_reps in _RUNNER_CACHE:
        return _RUNNER_CACHE[n_reps]
    import jax
    from jax.sharding import Mesh, PartitionSpec
    from jax.experimental.shard_map import shard_map
    from concourse.bass2jax import (_bass_exec_p, install_neuronx_cc_hook,
                                    partition_id_tensor)

    nc = _build_nc(n_reps)
    install_neuronx_cc_hook()
    partition_name = nc.partition_id_tensor.name if nc.partition_id_tensor else None
    in_names, out_names, out_avals, zero_outs = [], [], [], []
    for alloc in nc.m.functions[0].allocations:
        if not isinstance(alloc, mybir.MemoryLocationSet):
            continue
        name = alloc.memorylocations[0].name
        if alloc.kind == "ExternalInput":
            if name != partition_name:
                in_names.append(name)
        elif alloc.kind == "ExternalOutput":
            out_names.append(name)
            shape = tuple(alloc.tensor_shape)
            dtype = mybir.dt.np(alloc.dtype)
            out_avals.append(jax.core.ShapedArray(shape, dtype))
            zero_outs.append(np.zeros(shape, dtype))
    n_params = len(in_names)
    all_in_names = list(in_names) + out_names
    if partition_name is not None:
        all_in_names.append(partition_name)

    def _kernel_body(*args):
        operands = list(args)
        if partition_name is not None:
            operands.append(partition_id_tensor())
        outs = _bass_exec_p.bind(
            *operands,
            out_avals=tuple(out_avals),
            in_names=tuple(all_in_names),
            out_names=tuple(out_names),
            lowering_input_output_aliases=(),
            sim_require_finite=True,
            sim_require_nnan=True,
            nc=nc,
        )
        return tuple(outs)

    devices = jax.devices()[:N_CORES]
    mesh = Mesh(np.asarray(devices), ("core",))
    spec = PartitionSpec("core")
    fn = jax.jit(shard_map(
        _kernel_body, mesh=mesh,
        in_specs=(spec,) * (n_params + len(out_names)),
        out_specs=(spec,) * len(out_names), check_rep=False))
    runner = (fn, in_names, out_names, zero_outs, mesh)
    _RUNNER_CACHE[n_reps] = runner
    return runner


def _run_core_maps(maps, n_reps: int = 1):
    import jax
    from jax.sharding import NamedSharding, PartitionSpec
    fn, in_names, out_names, zero_outs, mesh = _get_runner(n_reps)
    sh = NamedSharding(mesh, PartitionSpec("core"))
    args = [jax.device_put(
        np.concatenate([np.asarray(m[name]) for m in maps], axis=0), sh)
        for name in in_names]
    for z in zero_outs:
        args.append(jax.device_put(
            np.zeros((N_CORES * z.shape[0], *z.shape[1:]), z.dtype), sh))
    res = fn(*args)
    jax.block_until_ready(res)
    return np.asarray(res[0]).reshape(N_CORES, S, HID)


def kernel(hidden, wq, wkv, wgate, ape, sinks, wo,
           ratio=RATIO, head_dim=HD, rope_head_dim=RD, num_heads=NH):
    hidden = np.asarray(hidden, np.float32)
    maps = _prep_inputs(hidden, np.asarray(wq, np.float32),
                        np.asarray(wkv, np.float32),
                        np.asarray(wgate, np.float32),
                        np.asarray(ape, np.float32),
                        np.asarray(sinks, np.float32),
                        np.asarray(wo, np.float32))
    partials = _run_core_maps(maps)
    out = np.empty((B, S, HID), np.float32)
    for b in range(B):
        out[b] = partials[b * HPC:(b + 1) * HPC].astype(np.float64).sum(
            axis=0).astype(np.float32)
    return out


# revision 16
# speedup vs baseline: 1.0608x; 1.0608x over previous
"""DeepseekV4-style sparse attention on 8 Trainium2 cores (Bass/Tile).

Sharding: data-parallel over batch (2) x tensor-parallel over heads (16 -> 4
groups of 4).  Core c handles batch c//4 and heads [4*(c%4), 4*(c%4)+4).
Per-core partial outputs (attn_heads @ wo_rows) are summed on the host
(partials are emitted bf16; the host accumulates in f64).

The shared-KV work (kv/gate projection + gated pooling) is sharded across the
4 head-group cores of each batch: core g projects/pools only its quarter of
the sequence (1024 positions -> 256 windows), then an intra-batch AllGather
(replica groups [[0..3],[4..7]]) shares the finished bf16 kT / V tiles.  The
collective rides entirely behind the q projection (~120us of PE work).

Device layout notes:
  - host passes hidden TRANSPOSED ([HID, S]) and cast to bf16 so every matmul
    has its contraction dim on partitions with no on-device transposes
  - hq is the per-core kv/gate slab [HID, 1028]: columns = positions
    [1024g-4, 1024g+1024) (4-col halo for the cross-window pooling overlap;
    core g=0 gets zeros + a -30000 additive gate pad instead)
  - weights/tables load ONCE (outside the unrolled timing reps)
  - q is produced directly in qT layout [head_dim, t] (matmul lhsT = wq)
  - scores are computed transposed (S^T[w, q]); the softmax denominator is
    accumulated by an ALL-ONES [128,128] lhsT so it lands replicated on all
    128 PSUM partitions; 1/(den+esink) runs as ACT ln -> exp(-x)
  - causal structure: query chunk j (512 queries) sees w-chunks 0..j; only
    the diagonal chunk needs a mask, identical for every j (precomputed 0/1)
  - out-proj for chunk j is emitted after chunk j+1's attention so its
    attnT-wait never blocks the strictly-ordered PE queue
  - all RoPE math runs on partitions 64..127 so no op shifts partition bases
"""

import numpy as np
import ml_dtypes

import concourse.bass as bass
import concourse.mybir as mybir
import concourse.tile as tile
from concourse.bass import ts
from concourse.masks import make_identity

F32 = mybir.dt.float32
BF16 = mybir.dt.bfloat16
AF = mybir.ActivationFunctionType

# Problem constants (hardcoded per the harness contract).
B, S, HID, NH, HD, RD, RATIO = 2, 4096, 2048, 16, 128, 64, 4
THETA = 10000.0
NW = S // RATIO              # 1024 pooled windows
N_CORES = 8
HPC = 4                      # heads per core
CW = HPC * HD                # per-core q/wo width (512)
TCH = 512                    # t-chunk size
NCH = S // TCH               # 8 t-chunks
WCH = 128                    # w-chunk size
KCH = HID // 128             # 16 contraction chunks
SCALE = HD ** -0.5
NWL = NW // HPC              # 256 windows pooled locally per core
NCHL = (NWL * RATIO) // TCH  # 2 local kv/gate chunks
HQW = NWL * RATIO + RATIO    # 1028 (quarter + halo)
RG = [[0, 1, 2, 3], [4, 5, 6, 7]]   # intra-batch allgather groups

_PAIR_SWAP = [i ^ 1 for i in range(32)]


def _build_nc(n_reps: int = 1, split_waits: bool = True):
    nc = bass.Bass()
    dp = nc.declare_dram_parameter
    ht = dp("ht", [HID, S], BF16, isOutput=False)
    hq = dp("hq", [HID, HQW], BF16, isOutput=False)
    wq = dp("wq", [HID, CW], BF16, isOutput=False)
    wkv = dp("wkv", [HID, 2 * HD], BF16, isOutput=False)
    wg = dp("wg", [HID, 2 * HD], BF16, isOutput=False)
    wo = dp("wo", [CW, HID], BF16, isOutput=False)
    eape = dp("eape", [HD, 2 * RATIO], F32, isOutput=False)
    esink = dp("esink", [128, HPC], F32, isOutput=False)
    gpad = dp("gpad", [HD, RATIO], F32, isOutput=False)
    cosq = dp("cosq", [RD, S], BF16, isOutput=False)
    sinq = dp("sinq", [RD, S], BF16, isOutput=False)
    coskl = dp("coskl", [RD, NWL], BF16, isOutput=False)
    sinkl = dp("sinkl", [RD, NWL], BF16, isOutput=False)
    bandm = dp("bandm", [WCH, TCH], BF16, isOutput=False)
    out = dp("out", [S, HID], BF16, isOutput=True)

    with tile.TileContext(nc) as tc:
        with (
            tc.tile_pool(name="persist", bufs=1) as pp,
            tc.tile_pool(name="wts", bufs=1) as wts,
            tc.tile_pool(name="ccd", bufs=1, space="DRAM") as ccd,
        ):
            P = _preamble(nc, pp, wts, ccd, wq, wkv, wg, wo, eape, esink,
                          gpad, coskl, sinkl, bandm)
            # NOTE: collectives inside a hardware For_i loop desync the NRT
            # mesh ("rolled collectives"), so timing builds unroll the rep
            # python-side: every rep gets its own straight-line CC instance
            # (the same way the single-rep program executes repeatedly).
            for _ in range(n_reps):
                _rep(nc, tc, P, ht, hq, cosq, sinq, out)
    if split_waits:
        _split_multi_waits(nc)
    return nc


def _preamble(nc, pp, wts, ccd, wq, wkv, wg, wo, eape, esink, gpad,
              coskl, sinkl, bandm):
    """Weights, tables and constants: loaded once, live across reps."""
    P = {}
    P["qT"] = [pp.tile([128, S], BF16, tag=f"qT{m}", name=f"qT{m}")
               for m in range(HPC)]
    # local kv/gate slabs: the 4-col halo shift is baked into the slab
    P["kvlo"] = pp.tile([HD, NWL * RATIO], BF16, tag="kvlo", name="kvlo")
    P["kvhi"] = pp.tile([HD, NWL * RATIO], BF16, tag="kvhi", name="kvhi")
    P["glo"] = pp.tile([HD, NWL * RATIO], BF16, tag="glo", name="glo")
    P["ghi"] = pp.tile([HD, NWL * RATIO], BF16, tag="ghi", name="ghi")
    # k-rope tables live on partitions 64..127 (matching the rope rows)
    P["cosk_s"] = pp.tile([128, NWL], BF16, tag="cosk", name="cosk")
    P["sink_s"] = pp.tile([128, NWL], BF16, tag="sink", name="sink")
    P["eape_s"] = pp.tile([HD, 2 * RATIO], F32, tag="eape", name="eape")
    P["esink_s"] = pp.tile([128, HPC], F32, tag="esink", name="esink")
    P["gpad_s"] = pp.tile([HD, RATIO], F32, tag="gpad", name="gpad")
    P["bandm_s"] = pp.tile([WCH, TCH], BF16, tag="bandm", name="bandm")
    P["wo_s"] = pp.tile([HD, HPC, HID], BF16, tag="wo", name="wo")
    P["ones128"] = pp.tile([WCH, 128], BF16, tag="ones128", name="ones128")
    P["kT"] = pp.tile([HD, NW], BF16, tag="kT", name="kT")
    P["v_s"] = pp.tile([WCH, NW // WCH, HD], BF16, tag="v", name="v")
    P["pooledT"] = pp.tile([HD, NWL], F32, tag="pooledT", name="pooledT")
    P["pk"] = pp.tile([128, 2 * NWL], BF16, tag="pk", name="pk")
    P["ident"] = pp.tile([128, 128], F32, tag="ident", name="ident")
    P["wq_s"] = wts.tile([128, KCH, CW], BF16, tag="wq", name="wq")
    P["wkv_s"] = wts.tile([128, KCH, 2 * HD], BF16, tag="wkv", name="wkv")
    P["wg_s"] = wts.tile([128, KCH, 2 * HD], BF16, tag="wg", name="wg")
    P["ccin"] = ccd.tile([128, 2 * NWL], BF16, tag="ccin", name="ccin")
    P["ccout"] = ccd.tile([HPC * 128, 2 * NWL], BF16, tag="ccout",
                          name="ccout")

    nc.sync.dma_start(P["wq_s"][:], wq.rearrange("(k p) c -> p k c", p=128))
    nc.sync.dma_start(P["wkv_s"][:], wkv.rearrange("(k p) c -> p k c", p=128))
    nc.sync.dma_start(P["wg_s"][:], wg.rearrange("(k p) c -> p k c", p=128))
    nc.sync.dma_start(P["cosk_s"][RD:128, :], coskl[:])
    nc.sync.dma_start(P["sink_s"][RD:128, :], sinkl[:])
    nc.sync.dma_start(P["eape_s"][:], eape[:])
    nc.sync.dma_start(P["esink_s"][:], esink[:])
    nc.sync.dma_start(P["gpad_s"][:], gpad[:])
    nc.sync.dma_start(P["bandm_s"][:], bandm[:])
    nc.sync.dma_start(P["wo_s"][:], wo.rearrange("(h p) e -> p h e", p=HD))
    nc.vector.memset(P["ones128"][:], 1.0)
    make_identity(nc, P["ident"][:])
    return P


def _rep(nc, tc, P, ht, hq, cosq, sinq, out):
    qT, kvlo, kvhi, glo, ghi = (P["qT"], P["kvlo"], P["kvhi"], P["glo"],
                                P["ghi"])
    kT, v_s, pooledT, pk = P["kT"], P["v_s"], P["pooledT"], P["pk"]
    eape_s, esink_s, bandm_s, wo_s = (P["eape_s"], P["esink_s"], P["bandm_s"],
                                      P["wo_s"])
    ones128, ident, gpad_s = P["ones128"], P["ident"], P["gpad_s"]
    wq_s, wkv_s, wg_s = P["wq_s"], P["wkv_s"], P["wg_s"]
    ccin, ccout = P["ccin"], P["ccout"]

    # ---- stage A: local kv/gate proj + pooling + allgather, q proj ----
    with (
        tc.tile_pool(name="hslab", bufs=2) as hs,
        tc.tile_pool(name="hqslab", bufs=1) as hqp,
        tc.tile_pool(name="evict", bufs=3) as ev,
        tc.tile_pool(name="poolb", bufs=2) as pb,
        tc.tile_pool(name="poolacc", bufs=1) as pba,
        tc.tile_pool(name="psA", bufs=6, space="PSUM") as psA,
        tc.tile_pool(name="psT", bufs=2, space="PSUM") as psT,
    ):
        numer = pba.tile([HD, NWL], F32, tag="numer", name="numer")
        denom = pba.tile([HD, NWL], F32, tag="denom", name="denom")

        def _kv_gate_local(jc, hqs):
            # kv / gate projections of the local quarter (lo = features
            # 0:128 at halo offset 0, hi = 128:256 at offset RATIO); the
            # gate eviction folds in the additive pooling bias ape
            for dst, lo, wsrc in (
                (kvlo, True, wkv_s), (kvhi, False, wkv_s),
                (glo, True, wg_s), (ghi, False, wg_s),
            ):
                ps = psA.tile([128, TCH], F32, tag="psA", name="psA")
                col = slice(0, 128) if lo else slice(128, 256)
                base = jc * TCH + (0 if lo else RATIO)
                for k in range(KCH):
                    nc.tensor.matmul(ps[:], wsrc[:, k, col],
                                     hqs[:, k, base:base + TCH],
                                     start=(k == 0), stop=(k == KCH - 1))
                dsl = dst[:, ts(jc, TCH)]
                if dst is kvlo or dst is kvhi:
                    nc.scalar.copy(dsl, ps[:])
                else:
                    acol = slice(0, RATIO) if lo else slice(RATIO, 2 * RATIO)
                    nc.vector.tensor_add(
                        dsl.rearrange("d (w r) -> d w r", r=RATIO),
                        ps[:].rearrange("d (w r) -> d w r", r=RATIO),
                        eape_s[:, None, acol].to_broadcast(
                            [HD, TCH // RATIO, RATIO]))

        def _pool_chunk(jc):
            # pool the 128 windows [128jc, 128jc+128) from chunk jc
            wsl = ts(jc, TCH // RATIO)
            for half, (g_src, kv_src) in enumerate(
                ((glo, kvlo), (ghi, kvhi))
            ):
                csl = ts(jc, TCH)
                e = pb.tile([HD, TCH], BF16, tag="poole", name=f"poole{half}")
                nc.scalar.activation(e[:], g_src[:, csl], AF.Exp)
                ea = pb.tile([HD, TCH], F32, tag="poolea",
                             name=f"poolea{half}")
                nc.vector.tensor_mul(ea[:], e[:], kv_src[:, csl])
                for acc, src in ((denom, e), (numer, ea)):
                    s3 = src[:].rearrange("d (w r) -> d w r", r=RATIO)
                    nm = f"poolred{half}{1 if acc is numer else 0}"
                    ra = pb.tile([HD, TCH // RATIO], F32, tag="poolra",
                                 name=nm + "a")
                    nc.vector.tensor_add(ra[:], s3[:, :, 0], s3[:, :, 1])
                    rc = pb.tile([HD, TCH // RATIO], F32, tag="poolrc",
                                 name=nm + "c")
                    nc.vector.tensor_add(rc[:], s3[:, :, 2], s3[:, :, 3])
                    if half == 0:
                        nc.vector.tensor_add(acc[:, wsl], ra[:], rc[:])
                    else:
                        nc.vector.tensor_add(ra[:], ra[:], rc[:])
                        nc.vector.tensor_add(acc[:, wsl], acc[:, wsl], ra[:])

        def _pool_fin_local():
            # divide, RoPE, V-transpose -> pack tile pk = [kT_local | v_loc]
            rec = pb.tile([HD, NWL], F32, tag="poolrec", name="poolrec")
            nc.vector.reciprocal(rec[:], denom[:])
            nc.vector.tensor_mul(pooledT[:], numer[:], rec[:])
            nc.scalar.copy(pk[0:RD, 0:NWL], pooledT[0:RD, :])
            krb = pb.tile([128, NWL], BF16, tag="krope", name="krope")
            nc.scalar.copy(krb[RD:128, :], pooledT[RD:128, :])
            ksw = pb.tile([128, NWL], BF16, tag="kswap", name="kswap")
            nc.vector.stream_shuffle(ksw[RD:128, :], krb[RD:128, :],
                                     _PAIR_SWAP)
            nc.vector.tensor_mul(krb[RD:128, :], krb[RD:128, :],
                                 P["cosk_s"][RD:128, :])
            nc.vector.tensor_mul(ksw[RD:128, :], ksw[RD:128, :],
                                 P["sink_s"][RD:128, :])
            nc.vector.tensor_add(pk[RD:128, 0:NWL], krb[RD:128, :],
                                 ksw[RD:128, :])
            for wb in range(NWL // WCH):
                tp = psT.tile([128, 128], F32, tag="vtrans", name="vtrans")
                nc.tensor.transpose(tp[:], pooledT[:, ts(wb, 128)], ident[:])
                nc.scalar.copy(pk[:, NWL + wb * 128:NWL + (wb + 1) * 128],
                               tp[:])

        def _q_proj(j, hsl):
            tsl = ts(j, TCH)
            cq_t = ev.tile([128, TCH], BF16, tag="cosqc", name="cosqc")
            sq_t = ev.tile([128, TCH], BF16, tag="sinqc", name="sinqc")
            nc.sync.dma_start(cq_t[RD:128, :], cosq[:, tsl])
            nc.sync.dma_start(sq_t[RD:128, :], sinq[:, tsl])
            for m in range(HPC):
                ps = psA.tile([128, TCH], F32, tag="psA", name="psA")
                for k in range(KCH):
                    nc.tensor.matmul(ps[:], wq_s[:, k, ts(m, 128)],
                                     hsl[:, k, :], start=(k == 0),
                                     stop=(k == KCH - 1))
                # rows 0:64 pass through; rows 64:128 interleaved RoPE
                nc.scalar.copy(qT[m][0:RD, tsl], ps[0:RD, :])
                rb = ev.tile([128, TCH], BF16, tag="ropebuf", name="ropebuf")
                nc.scalar.copy(rb[RD:128, :], ps[RD:128, :])
                sw = ev.tile([128, TCH], BF16, tag="ropeswap", name="ropeswap")
                nc.vector.stream_shuffle(sw[RD:128, :], rb[RD:128, :],
                                         _PAIR_SWAP)
                t1 = ev.tile([128, TCH], BF16, tag="ropet1", name="ropet1")
                nc.vector.tensor_mul(t1[RD:128, :], rb[RD:128, :],
                                     cq_t[RD:128, :])
                t2 = ev.tile([128, TCH], BF16, tag="ropet2", name="ropet2")
                nc.vector.tensor_mul(t2[RD:128, :], sw[RD:128, :],
                                     sq_t[RD:128, :])
                nc.vector.tensor_add(qT[m][RD:128, tsl], t1[RD:128, :],
                                     t2[RD:128, :])

        # q chunk 0 first so the PE has work while the bigger hq slab lands
        hsl0 = hs.tile([128, KCH, TCH], BF16, tag="hslab", name="hslab")
        nc.sync.dma_start(
            hsl0[:], ht[:, ts(0, TCH)].rearrange("(k p) t -> p k t", p=128))
        hqs = hqp.tile([128, KCH, HQW], BF16, tag="hqslab", name="hqslab")
        nc.sync.dma_start(hqs[:], hq.rearrange("(k p) c -> p k c", p=128))
        _q_proj(0, hsl0)
        for jc in range(NCHL):
            _kv_gate_local(jc, hqs)
        # additive gate pad for the first window's halo (core g=0 only;
        # other cores receive zeros)
        nc.vector.tensor_add(glo[:, 0:RATIO], glo[:, 0:RATIO], gpad_s[:])
        for jc in range(NCHL):
            _pool_chunk(jc)
        _pool_fin_local()
        nc.sync.dma_start(ccin[:], pk[:])
        nc.gpsimd.collective_compute(
            "AllGather", mybir.AluOpType.bypass, replica_groups=RG,
            ins=[ccin[:]], outs=[ccout[:]])
        nc.sync.dma_start(
            kT[:].rearrange("p (g w) -> p g w", g=HPC),
            ccout[:, 0:NWL].rearrange("(g p) w -> p g w", g=HPC))
        nc.sync.dma_start(
            v_s[:].rearrange("p (g wb) d -> p g wb d", g=HPC),
            ccout[:, NWL:2 * NWL].rearrange("(g p) (wb d) -> p g wb d",
                                            g=HPC, d=HD))
        for j in range(1, NCH):
            hsl = hs.tile([128, KCH, TCH], BF16, tag="hslab", name="hslab")
            nc.sync.dma_start(
                hsl[:], ht[:, ts(j, TCH)].rearrange("(k p) t -> p k t", p=128))
            _q_proj(j, hsl)

    # ---- stage C: attention + output projection, per q-chunk ----
    with (
        tc.tile_pool(name="pt", bufs=4) as ptp,
        tc.tile_pool(name="att", bufs=2) as att,
        tc.tile_pool(name="osb", bufs=6) as osb,
        tc.tile_pool(name="psS", bufs=2, space="PSUM") as psS,
        tc.tile_pool(name="psO", bufs=2, space="PSUM") as pso,
        tc.tile_pool(name="psAcc", bufs=2, space="PSUM") as psacc,
    ):
        def _attn_chunk(j):
            tsl = ts(j, TCH)
            attnT = []
            for h in range(HPC):
                qsl = qT[h][:, tsl]
                den_ps = psacc.tile([128, TCH], F32, tag="den", name="den")
                acc_ps = psacc.tile([128, TCH], F32, tag="acc", name="acc")
                # score matmuls run one w-chunk ahead of the exp/sum/PV
                # consumers so PE never sits behind the ACT exp
                s_tiles = {}

                def _score(wc, s_tiles=s_tiles, qsl=qsl):
                    sp = psS.tile([WCH, TCH], F32, tag="s", name="s")
                    nc.tensor.matmul(sp[:], kT[:, ts(wc, WCH)], qsl,
                                     start=True, stop=True)
                    s_tiles[wc] = sp

                _score(0)
                for wc in range(j + 1):
                    if wc < j:
                        _score(wc + 1)
                    s_ps = s_tiles.pop(wc)
                    pt = ptp.tile([WCH, TCH], BF16, tag="pt", name="pt")
                    nc.scalar.activation(pt[:], s_ps[:], AF.Exp, scale=SCALE)
                    if wc == j:
                        ptm = ptp.tile([WCH, TCH], BF16, tag="ptm", name="ptm")
                        nc.vector.tensor_mul(ptm[:], pt[:], bandm_s[:])
                        pt = ptm
                    # denominator rides on an all-ones lhsT so it lands
                    # replicated across all 128 PSUM partitions
                    nc.tensor.matmul(den_ps[:], ones128[:], pt[:],
                                     start=(wc == 0), stop=(wc == j))
                    nc.tensor.matmul(acc_ps[:], v_s[:, wc, :], pt[:],
                                     start=(wc == 0), stop=(wc == j))
                # 1/(den+esink) via ACT ln->exp(-x): keeps the 3.4us DVE
                # reciprocal off the critical path (ACT is lightly loaded)
                lden = att.tile([128, TCH], F32, tag="lden", name="lden")
                nc.scalar.activation(lden[:], den_ps[:], AF.Ln,
                                     bias=esink_s[:, h:h + 1])
                rec_sb = att.tile([128, TCH], F32, tag="rec_sb", name="rec_sb")
                nc.scalar.activation(rec_sb[:], lden[:], AF.Exp, scale=-1.0)
                a_sb = att.tile([128, TCH], BF16, tag=f"attnT{h}",
                                name=f"attnT{h}")
                nc.vector.tensor_mul(a_sb[:], acc_ps[:], rec_sb[:])
                attnT.append(a_sb)
            return attnT

        def _out_proj(j, attnT):
            for tt in range(TCH // 128):
                for e in range(HID // TCH):
                    o_ps = pso.tile([128, TCH], F32, tag="o", name="o")
                    for h in range(HPC):
                        nc.tensor.matmul(o_ps[:], attnT[h][:, ts(tt, 128)],
                                         wo_s[:, h, ts(e, TCH)],
                                         start=(h == 0), stop=(h == HPC - 1))
                    o_sb = osb.tile([128, TCH], BF16, tag="o_sb", name="o_sb")
                    # alternate ACT/DVE so neither strict-FIFO queue carries
                    # all the PSUM->SBUF copies (measured best split)
                    if e % 2 == 0:
                        nc.scalar.copy(o_sb[:], o_ps[:])
                    else:
                        nc.vector.tensor_copy(o_sb[:], o_ps[:])
                    nc.sync.dma_start(
                        out[j * TCH + tt * 128:j * TCH + (tt + 1) * 128,
                            ts(e, TCH)], o_sb[:])

        # out-proj for a chunk is emitted AFTER the next chunk's attention so
        # its attnT-wait never blocks the later score matmuls in the
        # strictly-ordered PE queue (attnT tags are double-buffered).
        # Descending j: the small causal chunks run last, where the previous
        # chunk's pipelined out-proj fills their epilogue stalls.
        prev = None
        for j in range(NCH - 1, -1, -1):
            cur = (j, _attn_chunk(j))
            if prev is not None:
                _out_proj(*prev)
            prev = cur
        _out_proj(*prev)


_WS_CTR = [0]


def _split_multi_waits(nc):
    """This walrus build accepts at most ONE sync wait per instruction; hoist
    extras onto same-engine NOPs placed immediately before."""
    f = nc.m.functions[0]
    for blk in f.blocks:
        insts = blk.instructions
        if not any(i.sync_info is not None and len(i.sync_info.on_wait) > 1
                   for i in insts):
            continue
        new_list = []
        for inst in insts:
            si = inst.sync_info
            if si is not None and len(si.on_wait) > 1:
                waits = list(si.on_wait)
                for w in waits[:-1]:
                    _WS_CTR[0] += 1
                    new_list.append(mybir.InstNoOp(
                        name=f"waitsplit-{_WS_CTR[0]}",
                        engine=inst.engine,
                        bass_nofuse=True,
                        sync_info=mybir.SyncInfo(on_wait=[w], on_update=[])))
                inst.sync_info = mybir.SyncInfo(
                    on_wait=[waits[-1]], on_update=list(si.on_update))
            new_list.append(inst)
        blk.instructions = new_list


# ---------------------------------------------------------------------------
# host side
# ---------------------------------------------------------------------------

def _rope_tables(positions):
    half = RD // 2
    inv_freq = 1.0 / (THETA ** (np.arange(half, dtype=np.float64) / half))
    ang = positions[None, :].astype(np.float64) * inv_freq[:, None]  # [32, L]
    cos_t = np.repeat(np.cos(ang), 2, axis=0).astype(np.float32)
    sin_t = np.repeat(np.sin(ang), 2, axis=0).astype(np.float32)
    sin_t[0::2] *= -1.0                                  # a-rows get -sin
    return cos_t, sin_t


def _prep_inputs(hidden, wq, wkv, wgate, ape, sinks, wo):
    bf = ml_dtypes.bfloat16
    cosq_t, sinq_t = _rope_tables(np.arange(S))
    cosk_t, sink_t = _rope_tables(np.arange(NW) * RATIO)
    pw, ft = np.meshgrid(np.arange(WCH), np.arange(TCH), indexing="ij")
    band = (ft >= RATIO * pw + RATIO - 1).astype(np.float32)     # [WCH, TCH]
    eape = np.empty((HD, 2 * RATIO), np.float32)
    for r in range(RATIO):
        eape[:, r] = ape[r, :HD]
        eape[:, RATIO + r] = ape[r, HD:]
    maps = []
    for c in range(N_CORES):
        b, g = divmod(c, HPC)
        htb = np.ascontiguousarray(hidden[b].T).astype(bf)
        if g == 0:
            hqm = np.concatenate(
                [np.zeros((HID, RATIO), bf), htb[:, :NWL * RATIO]], axis=1)
            gp = np.full((HD, RATIO), -30000.0, np.float32)
        else:
            lo = g * NWL * RATIO - RATIO
            hqm = htb[:, lo:lo + HQW]
            gp = np.zeros((HD, RATIO), np.float32)
        maps.append({
            "ht": htb,
            "hq": np.ascontiguousarray(hqm),
            "wq": np.ascontiguousarray(wq[:, g * CW:(g + 1) * CW]).astype(bf),
            "wkv": wkv.astype(bf),
            "wg": wgate.astype(bf),
            "wo": np.ascontiguousarray(wo[g * CW:(g + 1) * CW, :]).astype(bf),
            "eape": eape,
            "esink": np.tile(
                np.exp(sinks[g * HPC:(g + 1) * HPC]).astype(
                    np.float32).reshape(1, HPC), (128, 1)),
            "gpad": gp,
            "cosq": cosq_t.astype(bf), "sinq": sinq_t.astype(bf),
            "coskl": np.ascontiguousarray(
                cosk_t[:, g * NWL:(g + 1) * NWL]).astype(bf),
            "sinkl": np.ascontiguousarray(
                sink_t[:, g * NWL:(g + 1) * NWL]).astype(bf),
            "bandm": band.astype(bf),
        })
    return maps


_RUNNER_CACHE = {}


def _get_runner(n_reps: int = 1):
    if n_reps in _RUNNER_CACHE:
        return _RUNNER_CACHE[n_reps]
    import jax
    from jax.sharding import Mesh, PartitionSpec
    from jax.experimental.shard_map import shard_map
    from concourse.bass2jax import (_bass_exec_p, install_neuronx_cc_hook,
                                    partition_id_tensor)

    nc = _build_nc(n_reps)
    install_neuronx_cc_hook()
    partition_name = nc.partition_id_tensor.name if nc.partition_id_tensor else None
    in_names, out_names, out_avals, zero_outs = [], [], [], []
    for alloc in nc.m.functions[0].allocations:
        if not isinstance(alloc, mybir.MemoryLocationSet):
            continue
        name = alloc.memorylocations[0].name
        if alloc.kind == "ExternalInput":
            if name != partition_name:
                in_names.append(name)
        elif alloc.kind == "ExternalOutput":
            out_names.append(name)
            shape = tuple(alloc.tensor_shape)
            dtype = mybir.dt.np(alloc.dtype)
            out_avals.append(jax.core.ShapedArray(shape, dtype))
            zero_outs.append(np.zeros(shape, dtype))
    n_params = len(in_names)
    all_in_names = list(in_names) + out_names
    if partition_name is not None:
        all_in_names.append(partition_name)

    def _kernel_body(*args):
        operands = list(args)
        if partition_name is not None:
            operands.append(partition_id_tensor())
        outs = _bass_exec_p.bind(
            *operands,
            out_avals=tuple(out_avals),
            in_names=tuple(all_in_names),
            out_names=tuple(out_names),
            lowering_input_output_aliases=(),
            sim_require_finite=True,
            sim_require_nnan=True,
            nc=nc,
        )
        return tuple(outs)

    devices = jax.devices()[:N_CORES]
    mesh = Mesh(np.asarray(devices), ("core",))
    spec = PartitionSpec("core")
    fn = jax.jit(shard_map(
        _kernel_body, mesh=mesh,
        in_specs=(spec,) * (n_params + len(out_names)),
        out_specs=(spec,) * len(out_names), check_rep=False))
    runner = (fn, in_names, out_names, zero_outs, mesh)
    _RUNNER_CACHE[n_reps] = runner
    return runner


def _run_core_maps(maps, n_reps: int = 1):
    import jax
    from jax.sharding import NamedSharding, PartitionSpec
    fn, in_names, out_names, zero_outs, mesh = _get_runner(n_reps)
    sh = NamedSharding(mesh, PartitionSpec("core"))
    args = [jax.device_put(
        np.concatenate([np.asarray(m[name]) for m in maps], axis=0), sh)
        for name in in_names]
    for z in zero_outs:
        args.append(jax.device_put(
            np.zeros((N_CORES * z.shape[0], *z.shape[1:]), z.dtype), sh))
    res = fn(*args)
    jax.block_until_ready(res)
    return np.asarray(res[0]).reshape(N_CORES, S, HID)


def kernel(hidden, wq, wkv, wgate, ape, sinks, wo,
           ratio=RATIO, head_dim=HD, rope_head_dim=RD, num_heads=NH):
    hidden = np.asarray(hidden, np.float32)
    maps = _prep_inputs(hidden, np.asarray(wq, np.float32),
                        np.asarray(wkv, np.float32),
                        np.asarray(wgate, np.float32),
                        np.asarray(ape, np.float32),
                        np.asarray(sinks, np.float32),
                        np.asarray(wo, np.float32))
    partials = _run_core_maps(maps)
    out = np.empty((B, S, HID), np.float32)
    for b in range(B):
        out[b] = partials[b * HPC:(b + 1) * HPC].astype(np.float64).sum(
            axis=0).astype(np.float32)
    return out


# revision 19
# speedup vs baseline: 3.1051x; 2.9272x over previous
"""DeepseekV4-style sparse attention on 8 Trainium2 cores (Bass/Tile).

Sharding: data-parallel over batch (2) x tensor-parallel over heads (16 -> 4
groups of 4).  Core c handles batch c//4 and heads [4*(c%4), 4*(c%4)+4).
Per-core partial outputs (attn_heads @ wo_rows) are summed on the host
(partials are emitted bf16; the host accumulates in f64).

The shared-KV work (kv/gate projection + gated pooling) is sharded across the
4 head-group cores of each batch: core g projects/pools only its quarter of
the sequence (1024 positions -> 256 windows), then an intra-batch AllGather
(replica groups [[0..3],[4..7]]) shares the finished bf16 kT / V tiles.  The
collective rides entirely behind the q projection (~120us of PE work).

Device layout notes:
  - host passes hidden TRANSPOSED ([HID, S]) and cast to bf16 so every matmul
    has its contraction dim on partitions with no on-device transposes
  - hq is the per-core kv/gate slab [HID, 1028]: columns = positions
    [1024g-4, 1024g+1024) (4-col halo for the cross-window pooling overlap;
    core g=0 gets zeros + a -30000 additive gate pad instead)
  - weights/tables load ONCE (outside the unrolled timing reps)
  - q is produced directly in qT layout [head_dim, t] (matmul lhsT = wq)
  - scores are computed transposed (S^T[w, q]); the softmax denominator is
    accumulated by an ALL-ONES [128,128] lhsT so it lands replicated on all
    128 PSUM partitions; 1/(den+esink) runs as ACT ln -> exp(-x)
  - causal structure: query chunk j (512 queries) sees w-chunks 0..j; only
    the diagonal chunk needs a mask, identical for every j (precomputed 0/1)
  - out-proj for chunk j is emitted after chunk j+1's attention so its
    attnT-wait never blocks the strictly-ordered PE queue
  - all RoPE math runs on partitions 64..127 so no op shifts partition bases
"""

import numpy as np
import ml_dtypes

import concourse.bass as bass
import concourse.mybir as mybir
import concourse.tile as tile
from concourse.bass import ts
from concourse.masks import make_identity

F32 = mybir.dt.float32
BF16 = mybir.dt.bfloat16
AF = mybir.ActivationFunctionType

# Problem constants (hardcoded per the harness contract).
B, S, HID, NH, HD, RD, RATIO = 2, 4096, 2048, 16, 128, 64, 4
THETA = 10000.0
NW = S // RATIO              # 1024 pooled windows
N_CORES = 8
HPC = 4                      # heads per core
CW = HPC * HD                # per-core q/wo width (512)
TCH = 512                    # t-chunk size
NCH = S // TCH               # 8 t-chunks
WCH = 128                    # w-chunk size
KCH = HID // 128             # 16 contraction chunks
SCALE = HD ** -0.5
NWL = NW // HPC              # 256 windows pooled locally per core
NCHL = (NWL * RATIO) // TCH  # 2 local kv/gate chunks
HQW = NWL * RATIO + RATIO    # 1028 (quarter + halo)
RG = [[0, 1, 2, 3], [4, 5, 6, 7]]   # intra-batch allgather groups

_PAIR_SWAP = [i ^ 1 for i in range(32)]


def _build_nc(n_reps: int = 1, split_waits: bool = True):
    nc = bass.Bass()
    dp = nc.declare_dram_parameter
    ht = dp("ht", [HID, S], BF16, isOutput=False)
    hq = dp("hq", [HID, HQW], BF16, isOutput=False)
    wq = dp("wq", [HID, CW], BF16, isOutput=False)
    wkv = dp("wkv", [HID, 2 * HD], BF16, isOutput=False)
    wg = dp("wg", [HID, 2 * HD], BF16, isOutput=False)
    wo = dp("wo", [CW, HID], BF16, isOutput=False)
    eape = dp("eape", [HD, 2 * RATIO], F32, isOutput=False)
    esink = dp("esink", [128, HPC], F32, isOutput=False)
    gpad = dp("gpad", [HD, RATIO], F32, isOutput=False)
    cosq = dp("cosq", [RD, S], BF16, isOutput=False)
    sinq = dp("sinq", [RD, S], BF16, isOutput=False)
    coskl = dp("coskl", [RD, NWL], BF16, isOutput=False)
    sinkl = dp("sinkl", [RD, NWL], BF16, isOutput=False)
    bandm = dp("bandm", [WCH, TCH], BF16, isOutput=False)
    out = dp("out", [S, HID], BF16, isOutput=True)

    with tile.TileContext(nc) as tc:
        with (
            tc.tile_pool(name="persist", bufs=1) as pp,
            tc.tile_pool(name="wts", bufs=1) as wts,
            tc.tile_pool(name="ccd", bufs=1, space="DRAM") as ccd,
        ):
            P = _preamble(nc, pp, wts, ccd, wq, wkv, wg, wo, eape, esink,
                          gpad, coskl, sinkl, bandm)
            # NOTE: collectives inside a hardware For_i loop desync the NRT
            # mesh ("rolled collectives"), so timing builds unroll the rep
            # python-side: every rep gets its own straight-line CC instance
            # (the same way the single-rep program executes repeatedly).
            for _ in range(n_reps):
                _rep(nc, tc, P, ht, hq, cosq, sinq, out)
    if split_waits:
        _split_multi_waits(nc)
    return nc


def _preamble(nc, pp, wts, ccd, wq, wkv, wg, wo, eape, esink, gpad,
              coskl, sinkl, bandm):
    """Weights, tables and constants: loaded once, live across reps."""
    P = {}
    P["qT"] = [pp.tile([128, S], BF16, tag=f"qT{m}", name=f"qT{m}")
               for m in range(HPC)]
    # local kv/gate slabs: the 4-col halo shift is baked into the slab
    P["kvlo"] = pp.tile([HD, NWL * RATIO], BF16, tag="kvlo", name="kvlo")
    P["kvhi"] = pp.tile([HD, NWL * RATIO], BF16, tag="kvhi", name="kvhi")
    P["glo"] = pp.tile([HD, NWL * RATIO], BF16, tag="glo", name="glo")
    P["ghi"] = pp.tile([HD, NWL * RATIO], BF16, tag="ghi", name="ghi")
    # k-rope tables live on partitions 64..127 (matching the rope rows)
    P["cosk_s"] = pp.tile([128, NWL], BF16, tag="cosk", name="cosk")
    P["sink_s"] = pp.tile([128, NWL], BF16, tag="sink", name="sink")
    P["eape_s"] = pp.tile([HD, 2 * RATIO], F32, tag="eape", name="eape")
    P["esink_s"] = pp.tile([128, HPC], F32, tag="esink", name="esink")
    P["gpad_s"] = pp.tile([HD, RATIO], F32, tag="gpad", name="gpad")
    P["bandm_s"] = pp.tile([WCH, TCH], BF16, tag="bandm", name="bandm")
    P["wo_s"] = pp.tile([HD, HPC, HID], BF16, tag="wo", name="wo")
    P["ones128"] = pp.tile([WCH, 128], BF16, tag="ones128", name="ones128")
    P["kT"] = pp.tile([HD, NW], BF16, tag="kT", name="kT")
    P["v_s"] = pp.tile([WCH, NW // WCH, HD], BF16, tag="v", name="v")
    P["pooledT"] = pp.tile([HD, NWL], F32, tag="pooledT", name="pooledT")
    P["pk"] = pp.tile([128, 2 * NWL], BF16, tag="pk", name="pk")
    P["ident"] = pp.tile([128, 128], F32, tag="ident", name="ident")
    P["wq_s"] = wts.tile([128, KCH, CW], BF16, tag="wq", name="wq")
    P["wkv_s"] = wts.tile([128, KCH, 2 * HD], BF16, tag="wkv", name="wkv")
    P["wg_s"] = wts.tile([128, KCH, 2 * HD], BF16, tag="wg", name="wg")
    P["ccin"] = ccd.tile([128, 2 * NWL], BF16, tag="ccin", name="ccin")
    P["ccout"] = ccd.tile([HPC * 128, 2 * NWL], BF16, tag="ccout",
                          name="ccout")

    nc.sync.dma_start(P["wq_s"][:], wq.rearrange("(k p) c -> p k c", p=128))
    nc.sync.dma_start(P["wkv_s"][:], wkv.rearrange("(k p) c -> p k c", p=128))
    nc.sync.dma_start(P["wg_s"][:], wg.rearrange("(k p) c -> p k c", p=128))
    nc.sync.dma_start(P["cosk_s"][RD:128, :], coskl[:])
    nc.sync.dma_start(P["sink_s"][RD:128, :], sinkl[:])
    nc.sync.dma_start(P["eape_s"][:], eape[:])
    nc.sync.dma_start(P["esink_s"][:], esink[:])
    nc.sync.dma_start(P["gpad_s"][:], gpad[:])
    nc.sync.dma_start(P["bandm_s"][:], bandm[:])
    nc.sync.dma_start(P["wo_s"][:], wo.rearrange("(h p) e -> p h e", p=HD))
    nc.vector.memset(P["ones128"][:], 1.0)
    make_identity(nc, P["ident"][:])
    return P


def _rep(nc, tc, P, ht, hq, cosq, sinq, out):
    qT, kvlo, kvhi, glo, ghi = (P["qT"], P["kvlo"], P["kvhi"], P["glo"],
                                P["ghi"])
    kT, v_s, pooledT, pk = P["kT"], P["v_s"], P["pooledT"], P["pk"]
    eape_s, esink_s, bandm_s, wo_s = (P["eape_s"], P["esink_s"], P["bandm_s"],
                                      P["wo_s"])
    ones128, ident, gpad_s = P["ones128"], P["ident"], P["gpad_s"]
    wq_s, wkv_s, wg_s = P["wq_s"], P["wkv_s"], P["wg_s"]
    ccin, ccout = P["ccin"], P["ccout"]

    # ---- stage A: local kv/gate proj + pooling + allgather, q proj ----
    with (
        tc.tile_pool(name="hslab", bufs=2) as hs,
        tc.tile_pool(name="hqslab", bufs=1) as hqp,
        tc.tile_pool(name="evict", bufs=3) as ev,
        tc.tile_pool(name="poolb", bufs=2) as pb,
        tc.tile_pool(name="poolacc", bufs=1) as pba,
        tc.tile_pool(name="psA", bufs=7, space="PSUM") as psA,
        tc.tile_pool(name="psT", bufs=1, space="PSUM") as psT,
    ):
        numer = pba.tile([HD, NWL], F32, tag="numer", name="numer")
        denom = pba.tile([HD, NWL], F32, tag="denom", name="denom")

        def _kv_gate_local(jc, hqs):
            # kv / gate projections of the local quarter (lo = features
            # 0:128 at halo offset 0, hi = 128:256 at offset RATIO); the
            # gate eviction folds in the additive pooling bias ape
            for dst, lo, wsrc in (
                (kvlo, True, wkv_s), (kvhi, False, wkv_s),
                (glo, True, wg_s), (ghi, False, wg_s),
            ):
                ps = psA.tile([128, TCH], F32, tag="psA", name="psA")
                col = slice(0, 128) if lo else slice(128, 256)
                base = jc * TCH + (0 if lo else RATIO)
                for k in range(KCH):
                    nc.tensor.matmul(ps[:], wsrc[:, k, col],
                                     hqs[:, k, base:base + TCH],
                                     start=(k == 0), stop=(k == KCH - 1))
                dsl = dst[:, ts(jc, TCH)]
                if dst is kvlo or dst is kvhi:
                    nc.scalar.copy(dsl, ps[:])
                else:
                    acol = slice(0, RATIO) if lo else slice(RATIO, 2 * RATIO)
                    nc.vector.tensor_add(
                        dsl.rearrange("d (w r) -> d w r", r=RATIO),
                        ps[:].rearrange("d (w r) -> d w r", r=RATIO),
                        eape_s[:, None, acol].to_broadcast(
                            [HD, TCH // RATIO, RATIO]))

        def _pool_chunk(jc):
            # pool the 128 windows [128jc, 128jc+128) from chunk jc
            wsl = ts(jc, TCH // RATIO)
            for half, (g_src, kv_src) in enumerate(
                ((glo, kvlo), (ghi, kvhi))
            ):
                csl = ts(jc, TCH)
                e = pb.tile([HD, TCH], BF16, tag="poole", name=f"poole{half}")
                nc.scalar.activation(e[:], g_src[:, csl], AF.Exp)
                ea = pb.tile([HD, TCH], F32, tag="poolea",
                             name=f"poolea{half}")
                nc.vector.tensor_mul(ea[:], e[:], kv_src[:, csl])
                for acc, src in ((denom, e), (numer, ea)):
                    s3 = src[:].rearrange("d (w r) -> d w r", r=RATIO)
                    nm = f"poolred{half}{1 if acc is numer else 0}"
                    ra = pb.tile([HD, TCH // RATIO], F32, tag="poolra",
                                 name=nm + "a")
                    nc.vector.tensor_add(ra[:], s3[:, :, 0], s3[:, :, 1])
                    rc = pb.tile([HD, TCH // RATIO], F32, tag="poolrc",
                                 name=nm + "c")
                    nc.vector.tensor_add(rc[:], s3[:, :, 2], s3[:, :, 3])
                    if half == 0:
                        nc.vector.tensor_add(acc[:, wsl], ra[:], rc[:])
                    else:
                        nc.vector.tensor_add(ra[:], ra[:], rc[:])
                        nc.vector.tensor_add(acc[:, wsl], acc[:, wsl], ra[:])

        def _pool_fin_local():
            # divide, RoPE, V-transpose -> pack tile pk = [kT_local | v_loc]
            rec = pb.tile([HD, NWL], F32, tag="poolrec", name="poolrec")
            nc.vector.reciprocal(rec[:], denom[:])
            nc.vector.tensor_mul(pooledT[:], numer[:], rec[:])
            nc.scalar.copy(pk[0:RD, 0:NWL], pooledT[0:RD, :])
            krb = pb.tile([128, NWL], BF16, tag="krope", name="krope")
            nc.scalar.copy(krb[RD:128, :], pooledT[RD:128, :])
            ksw = pb.tile([128, NWL], BF16, tag="kswap", name="kswap")
            nc.vector.stream_shuffle(ksw[RD:128, :], krb[RD:128, :],
                                     _PAIR_SWAP)
            nc.vector.tensor_mul(krb[RD:128, :], krb[RD:128, :],
                                 P["cosk_s"][RD:128, :])
            nc.vector.tensor_mul(ksw[RD:128, :], ksw[RD:128, :],
                                 P["sink_s"][RD:128, :])
            nc.vector.tensor_add(pk[RD:128, 0:NWL], krb[RD:128, :],
                                 ksw[RD:128, :])
            for wb in range(NWL // WCH):
                tp = psT.tile([128, 128], F32, tag="vtrans", name="vtrans")
                nc.tensor.transpose(tp[:], pooledT[:, ts(wb, 128)], ident[:])
                nc.scalar.copy(pk[:, NWL + wb * 128:NWL + (wb + 1) * 128],
                               tp[:])

        def _q_proj(j, hsl):
            tsl = ts(j, TCH)
            cq_t = ev.tile([128, TCH], BF16, tag="cosqc", name="cosqc")
            sq_t = ev.tile([128, TCH], BF16, tag="sinqc", name="sinqc")
            nc.sync.dma_start(cq_t[RD:128, :], cosq[:, tsl])
            nc.sync.dma_start(sq_t[RD:128, :], sinq[:, tsl])
            for m in range(HPC):
                ps = psA.tile([128, TCH], F32, tag="psA", name="psA")
                for k in range(KCH):
                    nc.tensor.matmul(ps[:], wq_s[:, k, ts(m, 128)],
                                     hsl[:, k, :], start=(k == 0),
                                     stop=(k == KCH - 1))
                # rows 0:64 pass through; rows 64:128 interleaved RoPE
                nc.scalar.copy(qT[m][0:RD, tsl], ps[0:RD, :])
                rb = ev.tile([128, TCH], BF16, tag="ropebuf", name="ropebuf")
                nc.scalar.copy(rb[RD:128, :], ps[RD:128, :])
                sw = ev.tile([128, TCH], BF16, tag="ropeswap", name="ropeswap")
                nc.vector.stream_shuffle(sw[RD:128, :], rb[RD:128, :],
                                         _PAIR_SWAP)
                t1 = ev.tile([128, TCH], BF16, tag="ropet1", name="ropet1")
                nc.vector.tensor_mul(t1[RD:128, :], rb[RD:128, :],
                                     cq_t[RD:128, :])
                t2 = ev.tile([128, TCH], BF16, tag="ropet2", name="ropet2")
                nc.vector.tensor_mul(t2[RD:128, :], sw[RD:128, :],
                                     sq_t[RD:128, :])
                nc.vector.tensor_add(qT[m][RD:128, tsl], t1[RD:128, :],
                                     t2[RD:128, :])

        # q chunk 0 first so the PE has work while the bigger hq slab lands
        def _slab_dma(dst, src_cols):
            # split across two dma_starts so the slab rides two DMA queues
            half = KCH // 2
            r = src_cols.rearrange("(k p) t -> p k t", p=128)
            nc.sync.dma_start(dst[:, 0:half, :], r[:, 0:half, :])
            nc.sync.dma_start(dst[:, half:KCH, :], r[:, half:KCH, :])

        hsl0 = hs.tile([128, KCH, TCH], BF16, tag="hslab", name="hslab")
        _slab_dma(hsl0, ht[:, ts(0, TCH)])
        hqs = hqp.tile([128, KCH, HQW], BF16, tag="hqslab", name="hqslab")
        _slab_dma(hqs, hq[:, :])
        _q_proj(0, hsl0)
        for jc in range(NCHL):
            _kv_gate_local(jc, hqs)
        # additive gate pad for the first window's halo (core g=0 only;
        # other cores receive zeros)
        nc.vector.tensor_add(glo[:, 0:RATIO], glo[:, 0:RATIO], gpad_s[:])
        for jc in range(NCHL):
            _pool_chunk(jc)
        _pool_fin_local()
        nc.sync.dma_start(ccin[:], pk[:])
        nc.gpsimd.collective_compute(
            "AllGather", mybir.AluOpType.bypass, replica_groups=RG,
            ins=[ccin[:]], outs=[ccout[:]])
        nc.sync.dma_start(
            kT[:].rearrange("p (g w) -> p g w", g=HPC),
            ccout[:, 0:NWL].rearrange("(g p) w -> p g w", g=HPC))
        nc.sync.dma_start(
            v_s[:].rearrange("p (g wb) d -> p g wb d", g=HPC),
            ccout[:, NWL:2 * NWL].rearrange("(g p) (wb d) -> p g wb d",
                                            g=HPC, d=HD))
        for j in range(1, NCH):
            hsl = hs.tile([128, KCH, TCH], BF16, tag="hslab", name="hslab")
            _slab_dma(hsl, ht[:, ts(j, TCH)])
            _q_proj(j, hsl)

    # ---- stage C: attention + output projection, per q-chunk ----
    with (
        tc.tile_pool(name="pt", bufs=4) as ptp,
        tc.tile_pool(name="att", bufs=2) as att,
        tc.tile_pool(name="osb", bufs=6) as osb,
        tc.tile_pool(name="psS", bufs=2, space="PSUM") as psS,
        tc.tile_pool(name="psO", bufs=2, space="PSUM") as pso,
        tc.tile_pool(name="psAcc", bufs=2, space="PSUM") as psacc,
    ):
        def _attn_chunk(j):
            tsl = ts(j, TCH)
            attnT = []
            for h in range(HPC):
                qsl = qT[h][:, tsl]
                den_ps = psacc.tile([128, TCH], F32, tag="den", name="den")
                acc_ps = psacc.tile([128, TCH], F32, tag="acc", name="acc")
                # score matmuls run one w-chunk ahead of the exp/sum/PV
                # consumers so PE never sits behind the ACT exp
                s_tiles = {}

                def _score(wc, s_tiles=s_tiles, qsl=qsl):
                    sp = psS.tile([WCH, TCH], F32, tag="s", name="s")
                    nc.tensor.matmul(sp[:], kT[:, ts(wc, WCH)], qsl,
                                     start=True, stop=True)
                    s_tiles[wc] = sp

                _score(0)
                for wc in range(j + 1):
                    if wc < j:
                        _score(wc + 1)
                    s_ps = s_tiles.pop(wc)
                    pt = ptp.tile([WCH, TCH], BF16, tag="pt", name="pt")
                    nc.scalar.activation(pt[:], s_ps[:], AF.Exp, scale=SCALE)
                    if wc == j:
                        ptm = ptp.tile([WCH, TCH], BF16, tag="ptm", name="ptm")
                        nc.vector.tensor_mul(ptm[:], pt[:], bandm_s[:])
                        pt = ptm
                    # denominator rides on an all-ones lhsT so it lands
                    # replicated across all 128 PSUM partitions
                    nc.tensor.matmul(den_ps[:], ones128[:], pt[:],
                                     start=(wc == 0), stop=(wc == j))
                    nc.tensor.matmul(acc_ps[:], v_s[:, wc, :], pt[:],
                                     start=(wc == 0), stop=(wc == j))
                # 1/(den+esink) via ACT ln->exp(-x): keeps the 3.4us DVE
                # reciprocal off the critical path (ACT is lightly loaded)
                lden = att.tile([128, TCH], F32, tag="lden", name="lden")
                nc.scalar.activation(lden[:], den_ps[:], AF.Ln,
                                     bias=esink_s[:, h:h + 1])
                rec_sb = att.tile([128, TCH], F32, tag="rec_sb", name="rec_sb")
                nc.scalar.activation(rec_sb[:], lden[:], AF.Exp, scale=-1.0)
                a_sb = att.tile([128, TCH], BF16, tag=f"attnT{h}",
                                name=f"attnT{h}")
                nc.vector.tensor_mul(a_sb[:], acc_ps[:], rec_sb[:])
                attnT.append(a_sb)
            return attnT

        def _out_proj(j, attnT):
            for tt in range(TCH // 128):
                for e in range(HID // TCH):
                    o_ps = pso.tile([128, TCH], F32, tag="o", name="o")
                    for h in range(HPC):
                        nc.tensor.matmul(o_ps[:], attnT[h][:, ts(tt, 128)],
                                         wo_s[:, h, ts(e, TCH)],
                                         start=(h == 0), stop=(h == HPC - 1))
                    o_sb = osb.tile([128, TCH], BF16, tag="o_sb", name="o_sb")
                    # alternate ACT/DVE so neither strict-FIFO queue carries
                    # all the PSUM->SBUF copies (measured best split)
                    if e % 2 == 0:
                        nc.scalar.copy(o_sb[:], o_ps[:])
                    else:
                        nc.vector.tensor_copy(o_sb[:], o_ps[:])
                    nc.sync.dma_start(
                        out[j * TCH + tt * 128:j * TCH + (tt + 1) * 128,
                            ts(e, TCH)], o_sb[:])

        # out-proj for a chunk is emitted AFTER the next chunk's attention so
        # its attnT-wait never blocks the later score matmuls in the
        # strictly-ordered PE queue (attnT tags are double-buffered).
        # Descending j: the small causal chunks run last, where the previous
        # chunk's pipelined out-proj fills their epilogue stalls.
        prev = None
        for j in range(NCH - 1, -1, -1):
            cur = (j, _attn_chunk(j))
            if prev is not None:
                _out_proj(*prev)
            prev = cur
        _out_proj(*prev)


_WS_CTR = [0]


def _split_multi_waits(nc):
    """This walrus build accepts at most ONE sync wait per instruction; hoist
    extras onto same-engine NOPs placed immediately before."""
    f = nc.m.functions[0]
    for blk in f.blocks:
        insts = blk.instructions
        if not any(i.sync_info is not None and len(i.sync_info.on_wait) > 1
                   for i in insts):
            continue
        new_list = []
        for inst in insts:
            si = inst.sync_info
            if si is not None and len(si.on_wait) > 1:
                waits = list(si.on_wait)
                for w in waits[:-1]:
                    _WS_CTR[0] += 1
                    new_list.append(mybir.InstNoOp(
                        name=f"waitsplit-{_WS_CTR[0]}",
                        engine=inst.engine,
                        bass_nofuse=True,
                        sync_info=mybir.SyncInfo(on_wait=[w], on_update=[])))
                inst.sync_info = mybir.SyncInfo(
                    on_wait=[waits[-1]], on_update=list(si.on_update))
            new_list.append(inst)
        blk.instructions = new_list


# ---------------------------------------------------------------------------
# host side
# ---------------------------------------------------------------------------

def _rope_tables(positions):
    half = RD // 2
    inv_freq = 1.0 / (THETA ** (np.arange(half, dtype=np.float64) / half))
    ang = positions[None, :].astype(np.float64) * inv_freq[:, None]  # [32, L]
    cos_t = np.repeat(np.cos(ang), 2, axis=0).astype(np.float32)
    sin_t = np.repeat(np.sin(ang), 2, axis=0).astype(np.float32)
    sin_t[0::2] *= -1.0                                  # a-rows get -sin
    return cos_t, sin_t


def _prep_inputs(hidden, wq, wkv, wgate, ape, sinks, wo):
    bf = ml_dtypes.bfloat16
    cosq_t, sinq_t = _rope_tables(np.arange(S))
    cosk_t, sink_t = _rope_tables(np.arange(NW) * RATIO)
    pw, ft = np.meshgrid(np.arange(WCH), np.arange(TCH), indexing="ij")
    band = (ft >= RATIO * pw + RATIO - 1).astype(np.float32)     # [WCH, TCH]
    eape = np.empty((HD, 2 * RATIO), np.float32)
    for r in range(RATIO):
        eape[:, r] = ape[r, :HD]
        eape[:, RATIO + r] = ape[r, HD:]
    maps = []
    for c in range(N_CORES):
        b, g = divmod(c, HPC)
        htb = np.ascontiguousarray(hidden[b].T).astype(bf)
        if g == 0:
            hqm = np.concatenate(
                [np.zeros((HID, RATIO), bf), htb[:, :NWL * RATIO]], axis=1)
            gp = np.full((HD, RATIO), -30000.0, np.float32)
        else:
            lo = g * NWL * RATIO - RATIO
            hqm = htb[:, lo:lo + HQW]
            gp = np.zeros((HD, RATIO), np.float32)
        maps.append({
            "ht": htb,
            "hq": np.ascontiguousarray(hqm),
            "wq": np.ascontiguousarray(wq[:, g * CW:(g + 1) * CW]).astype(bf),
            "wkv": wkv.astype(bf),
            "wg": wgate.astype(bf),
            "wo": np.ascontiguousarray(wo[g * CW:(g + 1) * CW, :]).astype(bf),
            "eape": eape,
            "esink": np.tile(
                np.exp(sinks[g * HPC:(g + 1) * HPC]).astype(
                    np.float32).reshape(1, HPC), (128, 1)),
            "gpad": gp,
            "cosq": cosq_t.astype(bf), "sinq": sinq_t.astype(bf),
            "coskl": np.ascontiguousarray(
                cosk_t[:, g * NWL:(g + 1) * NWL]).astype(bf),
            "sinkl": np.ascontiguousarray(
                sink_t[:, g * NWL:(g + 1) * NWL]).astype(bf),
            "bandm": band.astype(bf),
        })
    return maps


_RUNNER_CACHE = {}


def _get_runner(n_reps: int = 1):
    if n_reps in _RUNNER_CACHE:
        return _RUNNER_CACHE[n_reps]
    import jax
    from jax.sharding import Mesh, PartitionSpec
    from jax.experimental.shard_map import shard_map
    from concourse.bass2jax import (_bass_exec_p, install_neuronx_cc_hook,
                                    partition_id_tensor)

    nc = _build_nc(n_reps)
    install_neuronx_cc_hook()
    partition_name = nc.partition_id_tensor.name if nc.partition_id_tensor else None
    in_names, out_names, out_avals, zero_outs = [], [], [], []
    for alloc in nc.m.functions[0].allocations:
        if not isinstance(alloc, mybir.MemoryLocationSet):
            continue
        name = alloc.memorylocations[0].name
        if alloc.kind == "ExternalInput":
            if name != partition_name:
                in_names.append(name)
        elif alloc.kind == "ExternalOutput":
            out_names.append(name)
            shape = tuple(alloc.tensor_shape)
            dtype = mybir.dt.np(alloc.dtype)
            out_avals.append(jax.core.ShapedArray(shape, dtype))
            zero_outs.append(np.zeros(shape, dtype))
    n_params = len(in_names)
    all_in_names = list(in_names) + out_names
    if partition_name is not None:
        all_in_names.append(partition_name)

    def _kernel_body(*args):
        operands = list(args)
        if partition_name is not None:
            operands.append(partition_id_tensor())
        outs = _bass_exec_p.bind(
            *operands,
            out_avals=tuple(out_avals),
            in_names=tuple(all_in_names),
            out_names=tuple(out_names),
            lowering_input_output_aliases=(),
            sim_require_finite=True,
            sim_require_nnan=True,
            nc=nc,
        )
        return tuple(outs)

    devices = jax.devices()[:N_CORES]
    mesh = Mesh(np.asarray(devices), ("core",))
    spec = PartitionSpec("core")
    fn = jax.jit(shard_map(
        _kernel_body, mesh=mesh,
        in_specs=(spec,) * (n_params + len(out_names)),
        out_specs=(spec,) * len(out_names), check_rep=False))
    runner = (fn, in_names, out_names, zero_outs, mesh)
    _RUNNER_CACHE[n_reps] = runner
    return runner


def _run_core_maps(maps, n_reps: int = 1):
    import jax
    from jax.sharding import NamedSharding, PartitionSpec
    fn, in_names, out_names, zero_outs, mesh = _get_runner(n_reps)
    sh = NamedSharding(mesh, PartitionSpec("core"))
    args = [jax.device_put(
        np.concatenate([np.asarray(m[name]) for m in maps], axis=0), sh)
        for name in in_names]
    for z in zero_outs:
        args.append(jax.device_put(
            np.zeros((N_CORES * z.shape[0], *z.shape[1:]), z.dtype), sh))
    res = fn(*args)
    jax.block_until_ready(res)
    return np.asarray(res[0]).reshape(N_CORES, S, HID)


def kernel(hidden, wq, wkv, wgate, ape, sinks, wo,
           ratio=RATIO, head_dim=HD, rope_head_dim=RD, num_heads=NH):
    hidden = np.asarray(hidden, np.float32)
    maps = _prep_inputs(hidden, np.asarray(wq, np.float32),
                        np.asarray(wkv, np.float32),
                        np.asarray(wgate, np.float32),
                        np.asarray(ape, np.float32),
                        np.asarray(sinks, np.float32),
                        np.asarray(wo, np.float32))
    partials = _run_core_maps(maps)
    out = np.empty((B, S, HID), np.float32)
    for b in range(B):
        out[b] = partials[b * HPC:(b + 1) * HPC].astype(np.float64).sum(
            axis=0).astype(np.float32)
    return out
